# revision 4
# baseline (speedup 1.0000x reference)
"""AttentionSortNet Trainium2 kernel.

kernel(q, k, u_gumbel, topk) -> [bh, buckets, kv_buckets] f32

Sharding: bh (batch*heads) axis split across 8 NeuronCores, embarrassingly
data-parallel. Per core: stream q/k shards (2x 8MB), compute bucket means,
R = sq @ sk.T * dim**-0.5, gumbel perturbation, 8 sinkhorn iterations
(linear-space, mathematically identical to the reference's log-space version),
return exp-domain doubly-normalized matrices.
"""

import numpy as np
from contextlib import ExitStack

import concourse.bass as bass
import concourse.tile as tile
from concourse import bacc, mybir
from concourse.bass_utils import run_bass_kernel_spmd

FP32 = mybir.dt.float32
AF = mybir.ActivationFunctionType
ALU = mybir.AluOpType
AX = mybir.AxisListType

N_CORES = 8
BUCKET = 128  # rows per bucket (both q and kv side)
EPS = 1e-6
INV_T = 1.0 / 0.7
R_SCALE = 0.125  # DIM ** -0.5 for DIM=64
SINKHORN_ITER = 8


def _emit(ctx: ExitStack, tc: "tile.TileContext", aps: dict, bh_per: int, seq: int, dim: int):
    nc = tc.nc
    q, k, u, g, id2b, idb, out = (
        aps["q"], aps["k"], aps["u"], aps["g"], aps["id2b"], aps["idb"], aps["out"]
    )
    B = seq // BUCKET            # buckets (= kv_buckets)
    r0 = seq // 128              # rows of one bh handled per partition
    mh = r0 // 2                 # rows per accumulation half
    assert r0 % 2 == 0 and bh_per % 2 == 0
    n_pairs = bh_per // 2

    # ---- pools ----
    const_p = ctx.enter_context(tc.tile_pool(name="const", bufs=1))
    acc_p = ctx.enter_context(tc.tile_pool(name="acc", bufs=2 * bh_per))
    p2_p = ctx.enter_context(tc.tile_pool(name="p2", bufs=3))
    sqt_sb_p = ctx.enter_context(tc.tile_pool(name="sqt_sb", bufs=2 * bh_per))
    sqt_ps_p = ctx.enter_context(tc.tile_pool(name="sqt_ps", bufs=2, space="PSUM"))
    init_p = ctx.enter_context(tc.tile_pool(name="init", bufs=4))
    upair_p = ctx.enter_context(tc.tile_pool(name="upair", bufs=2))
    sink_sb_p = ctx.enter_context(tc.tile_pool(name="sink_sb", bufs=4))
    small_p = ctx.enter_context(tc.tile_pool(name="small", bufs=8))
    s_ps_p = ctx.enter_context(tc.tile_pool(name="s_ps", bufs=3, space="PSUM"))
    w_ps_p = ctx.enter_context(tc.tile_pool(name="w_ps", bufs=3, space="PSUM"))
    out_p = ctx.enter_context(tc.tile_pool(name="outsb", bufs=2))

    # ---- constants to SBUF (ACT-ring DMAs; stay off the SP ring used for bulk) ----
    eps_sb = const_p.tile([128, 1], FP32)
    nc.gpsimd.memset(eps_sb[:], EPS)
    g_sb = const_p.tile([128, B], FP32)
    nc.scalar.dma_start(g_sb[:], g)
    id2b_sb = const_p.tile([2 * B, 2 * B], FP32)
    nc.scalar.dma_start(id2b_sb[:], id2b)
    idb_sb = const_p.tile([B, B], FP32)
    nc.scalar.dma_start(idb_sb[:], idb)
    u_flat = u.rearrange("b i j -> (b i) j")
    u_tiles = []
    for c in range(n_pairs):
        u_t = upair_p.tile([2 * B, B], FP32)
        nc.scalar.dma_start(u_t[:], u_flat[2 * c * B:(2 * c + 2) * B, :])
        u_tiles.append(u_t)

    # ---- bulk streaming + bucket sums ----
    # row of bh-slice = p*r0 + h*mh + m  (p: partition, h: half, m: row in half)
    qv = q.rearrange("b (p h m) d -> b h p (m d)", p=128, h=2, m=mh)
    kv = k.rearrange("b (p h m) d -> b h p (m d)", p=128, h=2, m=mh)

    sqt_tiles = {}
    for bh in range(bh_per):
        for name, src in (("q", qv), ("k", kv)):
            t = acc_p.tile([128, mh * dim], FP32, tag="acc")
            nc.sync.dma_start(t[:], src[bh, 0])
            nc.gpsimd.dma_start(t[:], src[bh, 1], accum_op=ALU.add)
            # fold the remaining mh rows per partition: view (m d) -> d, m
            p2 = p2_p.tile([128, dim], FP32, tag="p2")
            nc.vector.reduce_sum(p2[:], t[:].rearrange("p (m d) -> p d m", m=mh, d=dim), axis=AX.X)
            # group partition pairs into buckets, scaled by 1/128 -> means, transposed
            sqt_ps = sqt_ps_p.tile([dim, B], FP32, tag="sqt_ps")
            nc.tensor.matmul(sqt_ps[:], lhsT=p2[:], rhs=g_sb[:], start=True, stop=True)
            sqt_sb = sqt_sb_p.tile([dim, B], FP32, tag="sqt_sb")
            nc.scalar.copy(sqt_sb[:], sqt_ps[:])
            sqt_tiles[(bh, name)] = sqt_sb

    # ---- per-pair: R matmul, gumbel init, sinkhorn ----
    out_flat = out.rearrange("b i j -> (b i) j")
    for c in range(n_pairs):
        bh0, bh1 = 2 * c, 2 * c + 1
        r_ps = s_ps_p.tile([2 * B, B], FP32, tag="s_ps")
        nc.tensor.matmul(r_ps[0:B, :], lhsT=sqt_tiles[(bh0, "q")][:],
                         rhs=sqt_tiles[(bh0, "k")][:], start=True, stop=True)
        nc.tensor.matmul(r_ps[B:2 * B, :], lhsT=sqt_tiles[(bh1, "q")][:],
                         rhs=sqt_tiles[(bh1, "k")][:], start=True, stop=True)

        # P0 = exp((ln(relu(R)*2^-3 + eps) - ln(-ln(u + eps) + eps)) / T)
        rr = init_p.tile([2 * B, B], FP32, tag="rr")
        nc.vector.tensor_scalar_max(rr[:], r_ps[:], 0.0)
        ln_r = init_p.tile([2 * B, B], FP32, tag="lnr")
        nc.scalar.activation(ln_r[:], rr[:], AF.Ln, bias=eps_sb[0:2 * B, :], scale=R_SCALE)
        t1 = init_p.tile([2 * B, B], FP32, tag="t1")
        nc.scalar.activation(t1[:], u_tiles[c][:], AF.Ln, bias=eps_sb[0:2 * B, :], scale=1.0)
        t3 = init_p.tile([2 * B, B], FP32, tag="t3")
        nc.scalar.activation(t3[:], t1[:], AF.Ln, bias=eps_sb[0:2 * B, :], scale=-1.0)
        d0 = init_p.tile([2 * B, B], FP32, tag="d0")
        nc.vector.tensor_sub(d0[:], ln_r[:], t3[:])
        s_cur = init_p.tile([2 * B, B], FP32, tag="s0")
        nc.scalar.activation(s_cur[:], d0[:], AF.Exp, scale=INV_T)

        for it in range(SINKHORN_ITER):
            # axis=2 (over j): rows of tall form
            s_c = small_p.tile([2 * B, 1], FP32, tag="s_c")
            nc.vector.reduce_sum(s_c[:], s_cur[:], axis=AX.X)
            rc = small_p.tile([2 * B, 1], FP32, tag="rc")
            nc.vector.reciprocal(rc[:], s_c[:])
            ssc = sink_sb_p.tile([2 * B, B], FP32, tag="ssc")
            nc.vector.tensor_scalar_mul(ssc[:], s_cur[:], rc[:])
            w_ps = w_ps_p.tile([B, 2 * B], FP32, tag="w_ps")
            nc.tensor.matmul(w_ps[:], lhsT=ssc[:], rhs=id2b_sb[:], start=True, stop=True)
            # axis=1 (over i): inner chunks of wide form
            s_r = small_p.tile([B, 2], FP32, tag="s_r")
            nc.vector.reduce_sum(s_r[:], w_ps[:].rearrange("j (h i) -> j h i", h=2), axis=AX.X)
            rw = small_p.tile([B, 2], FP32, tag="rw")
            nc.vector.reciprocal(rw[:], s_r[:])
            wsc = sink_sb_p.tile([B, 2 * B], FP32, tag="wsc")
            nc.vector.tensor_scalar_mul(wsc[:, 0:B], w_ps[:, 0:B], rw[:, 0:1])
            nc.scalar.activation(wsc[:, B:2 * B], w_ps[:, B:2 * B], AF.Copy, scale=rw[:, 1:2])
            s_ps = s_ps_p.tile([2 * B, B], FP32, tag="s_ps")
            nc.tensor.matmul(s_ps[:], lhsT=wsc[:], rhs=idb_sb[:], start=True, stop=True)
            s_cur = s_ps

        s_fin = out_p.tile([2 * B, B], FP32, tag="s_fin")
        nc.scalar.copy(s_fin[:], s_cur[:])
        nc.sync.dma_start(out_flat[2 * c * B:(2 * c + 2) * B, :], s_fin[:])


def build(bh_per: int, seq: int, dim: int):
    nc = bacc.Bacc(
        "TRN2",
        target_bir_lowering=False,
        debug=False,
        enable_asserts=True,
        num_devices=1,
    )
    B = seq // BUCKET
    aps = {}
    aps["q"] = nc.dram_tensor("q", (bh_per, seq, dim), FP32, kind="ExternalInput").ap()
    aps["k"] = nc.dram_tensor("k", (bh_per, seq, dim), FP32, kind="ExternalInput").ap()
    aps["u"] = nc.dram_tensor("u", (bh_per, B, B), FP32, kind="ExternalInput").ap()
    aps["g"] = nc.dram_tensor("g", (128, B), FP32, kind="ExternalInput").ap()
    aps["id2b"] = nc.dram_tensor("id2b", (2 * B, 2 * B), FP32, kind="ExternalInput").ap()
    aps["idb"] = nc.dram_tensor("idb", (B, B), FP32, kind="ExternalInput").ap()
    aps["out"] = nc.dram_tensor("out", (bh_per, B, B), FP32, kind="ExternalOutput").ap()
    with tile.TileContext(nc) as tc:
        with ExitStack() as ctx:
            _emit(ctx, tc, aps, bh_per, seq, dim)
    nc.compile()
    return nc


def make_consts(seq: int):
    B = seq // BUCKET
    r0 = seq // 128
    ppb = BUCKET // r0  # partitions per bucket
    g = np.zeros((128, B), dtype=np.float32)
    for p in range(128):
        g[p, p // ppb] = 1.0 / BUCKET
    id2b = np.eye(2 * B, dtype=np.float32)
    idb = np.eye(B, dtype=np.float32)
    return g, id2b, idb


_CACHE: dict = {}


def _get_nc(bh_per: int, seq: int, dim: int):
    key = (bh_per, seq, dim)
    if key not in _CACHE:
        _CACHE[key] = build(bh_per, seq, dim)
    return _CACHE[key]


def run(q, k, u_gumbel, topk=1, trace=False):
    q = np.ascontiguousarray(np.asarray(q), dtype=np.float32)
    k = np.ascontiguousarray(np.asarray(k), dtype=np.float32)
    u = np.ascontiguousarray(np.asarray(u_gumbel), dtype=np.float32)
    bh, seq, dim = q.shape
    per = bh // N_CORES
    nc = _get_nc(per, seq, dim)
    g, id2b, idb = make_consts(seq)
    in_maps = []
    for c in range(N_CORES):
        sl = slice(c * per, (c + 1) * per)
        in_maps.append({
            "q": q[sl], "k": k[sl], "u": u[sl],
            "g": g, "id2b": id2b, "idb": idb,
        })
    res = run_bass_kernel_spmd(nc, in_maps, core_ids=list(range(N_CORES)), trace=trace)
    out = np.concatenate([res.results[c]["out"] for c in range(N_CORES)], axis=0)
    return out, res


def kernel(q, k, u_gumbel, topk=1):
    out, _ = run(q, k, u_gumbel, topk)
    return out


# revision 7
# speedup vs baseline: 1.1152x; 1.1152x over previous
"""AttentionSortNet Trainium2 kernel.

kernel(q, k, u_gumbel, topk) -> [bh, buckets, kv_buckets] f32

Sharding: bh (batch*heads) axis split across 8 NeuronCores, embarrassingly
data-parallel. Per core: stream q/k shards (2x 8MB), compute bucket means,
R = sq @ sk.T * dim**-0.5, gumbel perturbation, 8 sinkhorn iterations
(linear-space, mathematically identical to the reference's log-space version),
return exp-domain doubly-normalized matrices.
"""

import numpy as np
from contextlib import ExitStack

import concourse.bass as bass
import concourse.tile as tile
from concourse import bacc, mybir
from concourse.bass_utils import run_bass_kernel_spmd

FP32 = mybir.dt.float32
AF = mybir.ActivationFunctionType
ALU = mybir.AluOpType
AX = mybir.AxisListType

N_CORES = 8
BUCKET = 128  # rows per bucket (both q and kv side)
EPS = 1e-6
INV_T = 1.0 / 0.7
R_SCALE = 0.125  # DIM ** -0.5 for DIM=64
SINKHORN_ITER = 8


def _emit(ctx: ExitStack, tc: "tile.TileContext", aps: dict, bh_per: int, seq: int, dim: int):
    nc = tc.nc
    q, k, u, g, id2b, idb, out = (
        aps["q"], aps["k"], aps["u"], aps["g"], aps["id2b"], aps["idb"], aps["out"]
    )
    B = seq // BUCKET            # buckets (= kv_buckets)
    r0 = seq // 128              # rows of one bh handled per partition
    mh = r0 // 2                 # rows per DMA half
    assert r0 % 2 == 0 and bh_per == 4 and B in (32, 64) and dim == 64
    S2 = 2 * B               # square side: partition=(h,i), free=(c,j)

    # ---- pools ----
    const_p = ctx.enter_context(tc.tile_pool(name="const", bufs=1))
    acc_p = ctx.enter_context(tc.tile_pool(name="acc", bufs=2 * bh_per))
    p2_p = ctx.enter_context(tc.tile_pool(name="p2", bufs=4))
    sqt_sb_p = ctx.enter_context(tc.tile_pool(name="sqt_sb", bufs=2 * bh_per))
    sqt_ps_p = ctx.enter_context(tc.tile_pool(name="sqt_ps", bufs=2, space="PSUM"))
    init_p = ctx.enter_context(tc.tile_pool(name="init", bufs=2))
    sink_sb_p = ctx.enter_context(tc.tile_pool(name="sink_sb", bufs=3))
    small_p = ctx.enter_context(tc.tile_pool(name="small", bufs=6))
    s_ps_p = ctx.enter_context(tc.tile_pool(name="s_ps", bufs=3, space="PSUM"))
    out_p = ctx.enter_context(tc.tile_pool(name="outsb", bufs=1))

    # ---- constants to SBUF (ACT-ring DMAs; stay off the SP ring used for bulk) ----
    eps_sb = const_p.tile([128, 1], FP32)
    nc.gpsimd.memset(eps_sb[:], EPS)
    g_sb = const_p.tile([128, B], FP32)
    nc.scalar.dma_start(g_sb[:], g)
    id2b_sb = const_p.tile([2 * B, 2 * B], FP32)
    nc.scalar.dma_start(id2b_sb[:], id2b)
    # u in "square" layout: partition = (h, i), free = (c, j), bh = 2c + h
    u_sq_dram = u.rearrange("(c h) i j -> h i c j", c=2, h=2)
    u_sq = const_p.tile([S2, S2], FP32)
    nc.scalar.dma_start(u_sq[:], u_sq_dram)

    # ---- bulk streaming + bucket sums ----
    # row of bh-slice = p*r0 + h*mh + m  (p: partition, h: half, m: row in half)
    qv = q.rearrange("b (p h m) d -> b h p (m d)", p=128, h=2, m=mh)
    kv = k.rearrange("b (p h m) d -> b h p (m d)", p=128, h=2, m=mh)

    sqt_tiles = {}
    for bh in range(bh_per):
        for name, src in (("q", qv), ("k", kv)):
            sqt_ps = sqt_ps_p.tile([dim, B], FP32, tag="sqt_ps")
            for h in range(2):
                t = acc_p.tile([128, mh * dim], FP32, tag="acc")
                nc.sync.dma_start(t[:], src[bh, h])
                # fold the mh rows per partition: view (m d) -> d, m
                p2 = p2_p.tile([128, dim], FP32, tag="p2")
                nc.vector.reduce_sum(
                    p2[:], t[:].rearrange("p (m d) -> p d m", m=mh, d=dim), axis=AX.X)
                # group partition pairs into buckets (scaled 1/128 -> means),
                # transposed to [d, b]; accumulate the two halves in PSUM
                nc.tensor.matmul(sqt_ps[:], lhsT=p2[:], rhs=g_sb[:],
                                 start=(h == 0), stop=(h == 1))
            sqt_sb = sqt_sb_p.tile([dim, B], FP32, tag="sqt_sb")
            nc.scalar.copy(sqt_sb[:], sqt_ps[:])
            sqt_tiles[(bh, name)] = sqt_sb

    # ---- R matmuls into the square [128=(h,i), 128=(c,j)] ----
    s_ps0 = s_ps_p.tile([S2, S2], FP32, tag="s_ps")
    for c in range(2):
        for h in range(2):
            bh = 2 * c + h
            nc.tensor.matmul(s_ps0[h * B:(h + 1) * B, c * B:(c + 1) * B],
                             lhsT=sqt_tiles[(bh, "q")][:],
                             rhs=sqt_tiles[(bh, "k")][:], start=True, stop=True)

    # ---- init: P0 = exp((ln(relu(R)*2^-3 + eps) - ln(-ln(u + eps) + eps)) / T) ----
    rr = init_p.tile([S2, S2], FP32, tag="rr")
    nc.vector.tensor_scalar_max(rr[:], s_ps0[:], 0.0)
    ln_r = init_p.tile([S2, S2], FP32, tag="lnr")
    nc.scalar.activation(ln_r[:], rr[:], AF.Ln, bias=eps_sb[0:S2, :], scale=R_SCALE)
    t1 = init_p.tile([S2, S2], FP32, tag="t1")
    nc.scalar.activation(t1[:], u_sq[:], AF.Ln, bias=eps_sb[0:S2, :], scale=1.0)
    t3 = init_p.tile([S2, S2], FP32, tag="t3")
    nc.scalar.activation(t3[:], t1[:], AF.Ln, bias=eps_sb[0:S2, :], scale=-1.0)
    d0 = init_p.tile([S2, S2], FP32, tag="d0")
    nc.vector.tensor_sub(d0[:], ln_r[:], t3[:])
    s_cur = init_p.tile([S2, S2], FP32, tag="s0")
    nc.scalar.activation(s_cur[:], d0[:], AF.Exp, scale=INV_T)

    # ---- sinkhorn: alternate tall [(h,i),(c,j)] / wide [(c,j),(h,i)] forms ----
    # In both forms the normalized axis is the inner free chunk; the scale
    # factor for free block x is per-partition scalar rs[:, x].
    for half_step in range(2 * SINKHORN_ITER):
        ss = small_p.tile([S2, 2], FP32, tag="ss")
        nc.vector.reduce_sum(ss[:], s_cur[:].rearrange("p (x j) -> p x j", x=2), axis=AX.X)
        rs = small_p.tile([S2, 2], FP32, tag="rs")
        nc.vector.reciprocal(rs[:], ss[:])
        ssc = sink_sb_p.tile([S2, S2], FP32, tag="ssc")
        nc.vector.tensor_scalar_mul(ssc[:, 0:B], s_cur[:, 0:B], rs[:, 0:1])
        nc.scalar.activation(ssc[:, B:2 * B], s_cur[:, B:2 * B], AF.Copy, scale=rs[:, 1:2])
        s_ps = s_ps_p.tile([S2, S2], FP32, tag="s_ps")
        nc.tensor.transpose(s_ps[:], ssc[:], id2b_sb[:])
        s_cur = s_ps

    # s_cur is tall again after an even number of transposes
    s_fin = out_p.tile([S2, S2], FP32, tag="s_fin")
    nc.scalar.copy(s_fin[:], s_cur[:])
    out_sq = out.rearrange("(c h) i j -> h i c j", c=2, h=2)
    nc.sync.dma_start(out_sq, s_fin[:])


def build(bh_per: int, seq: int, dim: int):
    nc = bacc.Bacc(
        "TRN2",
        target_bir_lowering=False,
        debug=False,
        enable_asserts=True,
        num_devices=1,
    )
    B = seq // BUCKET
    aps = {}
    aps["q"] = nc.dram_tensor("q", (bh_per, seq, dim), FP32, kind="ExternalInput").ap()
    aps["k"] = nc.dram_tensor("k", (bh_per, seq, dim), FP32, kind="ExternalInput").ap()
    aps["u"] = nc.dram_tensor("u", (bh_per, B, B), FP32, kind="ExternalInput").ap()
    aps["g"] = nc.dram_tensor("g", (128, B), FP32, kind="ExternalInput").ap()
    aps["id2b"] = nc.dram_tensor("id2b", (2 * B, 2 * B), FP32, kind="ExternalInput").ap()
    aps["idb"] = nc.dram_tensor("idb", (B, B), FP32, kind="ExternalInput").ap()
    aps["out"] = nc.dram_tensor("out", (bh_per, B, B), FP32, kind="ExternalOutput").ap()
    with tile.TileContext(nc) as tc:
        with ExitStack() as ctx:
            _emit(ctx, tc, aps, bh_per, seq, dim)
    nc.compile()
    return nc


def make_consts(seq: int):
    B = seq // BUCKET
    r0 = seq // 128
    ppb = BUCKET // r0  # partitions per bucket
    g = np.zeros((128, B), dtype=np.float32)
    for p in range(128):
        g[p, p // ppb] = 1.0 / BUCKET
    id2b = np.eye(2 * B, dtype=np.float32)
    idb = np.eye(B, dtype=np.float32)
    return g, id2b, idb


_CACHE: dict = {}


def _get_nc(bh_per: int, seq: int, dim: int):
    key = (bh_per, seq, dim)
    if key not in _CACHE:
        _CACHE[key] = build(bh_per, seq, dim)
    return _CACHE[key]


def run(q, k, u_gumbel, topk=1, trace=False):
    q = np.ascontiguousarray(np.asarray(q), dtype=np.float32)
    k = np.ascontiguousarray(np.asarray(k), dtype=np.float32)
    u = np.ascontiguousarray(np.asarray(u_gumbel), dtype=np.float32)
    bh, seq, dim = q.shape
    per = bh // N_CORES
    nc = _get_nc(per, seq, dim)
    g, id2b, idb = make_consts(seq)
    in_maps = []
    for c in range(N_CORES):
        sl = slice(c * per, (c + 1) * per)
        in_maps.append({
            "q": q[sl], "k": k[sl], "u": u[sl],
            "g": g, "id2b": id2b, "idb": idb,
        })
    res = run_bass_kernel_spmd(nc, in_maps, core_ids=list(range(N_CORES)), trace=trace)
    out = np.concatenate([res.results[c]["out"] for c in range(N_CORES)], axis=0)
    return out, res


def kernel(q, k, u_gumbel, topk=1):
    out, _ = run(q, k, u_gumbel, topk)
    return out


# revision 9
# speedup vs baseline: 1.4814x; 1.3284x over previous
"""AttentionSortNet Trainium2 kernel.

kernel(q, k, u_gumbel, topk) -> [bh, buckets, kv_buckets] f32

Sharding: bh (batch*heads) axis split across 8 NeuronCores, embarrassingly
data-parallel. Per core (bh_per=4): stream q/k shards (16MB, HWDGE), bucket
sums via a contiguous halving add-tree (DVE, + one gpsimd level), a small
grouping matmul produces transposed bucket means, R = sq @ sk.T * dim**-0.5,
gumbel perturbation, then 8 sinkhorn iterations in *vector form*
(u = 1/(P0 v), v = 1/(P0^T u) via PE matvecs; mathematically the same
iteration as the reference's log-space normalization), final
P = diag(u) P0 diag(v).
"""

import numpy as np
from contextlib import ExitStack

import concourse.bass as bass
import concourse.tile as tile
from concourse import bacc, mybir
from concourse.bass_utils import run_bass_kernel_spmd

FP32 = mybir.dt.float32
AF = mybir.ActivationFunctionType
ALU = mybir.AluOpType
AX = mybir.AxisListType

N_CORES = 8
BUCKET = 128  # rows per bucket (both q and kv side)
EPS = 1e-6
INV_T = 1.0 / 0.7
R_SCALE = 0.125  # DIM ** -0.5 for DIM=64
SINKHORN_ITER = 8
GPSIMD_ADD1 = True  # first fold level of each tile's first half runs on gpsimd


def _emit(ctx: ExitStack, tc: "tile.TileContext", aps: dict, bh_per: int, seq: int, dim: int):
    nc = tc.nc
    q, k, u, g, id2b, masku, maskv, out = (
        aps["q"], aps["k"], aps["u"], aps["g"], aps["id2b"],
        aps["masku"], aps["maskv"], aps["out"])
    B = seq // BUCKET            # buckets (= kv_buckets)
    r0 = seq // 128              # rows of one bh handled per partition
    mh = r0 // 2                 # rows per DMA half
    assert r0 % 2 == 0 and bh_per == 4 and B in (32, 64) and dim == 64
    S2 = 2 * B                   # square side: partition=(h,i), free=(c,j)

    const_p = ctx.enter_context(tc.tile_pool(name="const", bufs=1))
    acc_p = ctx.enter_context(tc.tile_pool(name="acc", bufs=10))
    tree_p = ctx.enter_context(tc.tile_pool(name="tree", bufs=4))
    p2_p = ctx.enter_context(tc.tile_pool(name="p2", bufs=4))
    sqt_sb_p = ctx.enter_context(tc.tile_pool(name="sqt_sb", bufs=2 * bh_per))
    sqt_ps_p = ctx.enter_context(tc.tile_pool(name="sqt_ps", bufs=2, space="PSUM"))
    init_p = ctx.enter_context(tc.tile_pool(name="init", bufs=2))
    small_p = ctx.enter_context(tc.tile_pool(name="small", bufs=8))
    s_ps_p = ctx.enter_context(tc.tile_pool(name="s_ps", bufs=2, space="PSUM"))
    mv_ps_p = ctx.enter_context(tc.tile_pool(name="mv_ps", bufs=3, space="PSUM"))
    out_p = ctx.enter_context(tc.tile_pool(name="outsb", bufs=2))

    # ---- constants (ACT-ring DMAs; keep the SP ring for bulk) ----
    eps_sb = const_p.tile([128, 1], FP32)
    nc.gpsimd.memset(eps_sb[:], EPS)
    g_sb = const_p.tile([128, B], FP32)
    nc.scalar.dma_start(g_sb[:], g)
    id2b_sb = const_p.tile([S2, S2], FP32)
    nc.scalar.dma_start(id2b_sb[:], id2b)
    masku_sb = const_p.tile([S2, 4], FP32)
    nc.scalar.dma_start(masku_sb[:], masku)
    maskv_sb = const_p.tile([S2, 4], FP32)
    nc.scalar.dma_start(maskv_sb[:], maskv)
    # u in "square" layout: partition = (h, i), free = (c, j), bh = 2c + h
    u_sq = const_p.tile([S2, S2], FP32)
    nc.scalar.dma_start(u_sq[:], u.rearrange("(c h) i j -> h i c j", c=2, h=2))

    # gumbel path early (independent of q/k): t3 = ln(-ln(u + eps) + eps)
    t1 = init_p.tile([S2, S2], FP32, tag="t1")
    nc.scalar.activation(t1[:], u_sq[:], AF.Ln, bias=eps_sb[0:S2, :], scale=1.0)
    t3 = init_p.tile([S2, S2], FP32, tag="t3")
    nc.scalar.activation(t3[:], t1[:], AF.Ln, bias=eps_sb[0:S2, :], scale=-1.0)

    # ---- bulk streaming + bucket sums ----
    # row of bh-slice = p*r0 + h*mh + m  (p: partition, h: half, m: row in half)
    qv = q.rearrange("b (p h m) d -> b h p (m d)", p=128, h=2, m=mh)
    kv = k.rearrange("b (p h m) d -> b h p (m d)", p=128, h=2, m=mh)

    s_ps0 = s_ps_p.tile([S2, S2], FP32, tag="s_ps")   # R square
    ln_r = init_p.tile([S2, S2], FP32, tag="lnr")
    sqt_tiles = {}
    for bh in range(bh_per):
        for name, src in (("q", qv), ("k", kv)):
            halves = []
            for hf in range(2):
                t = acc_p.tile([128, mh * dim], FP32, tag="acc")
                nc.sync.dma_start(t[:], src[bh, hf])
                a1 = tree_p.tile([128, mh * dim // 2], FP32, tag="a1")
                eng = nc.gpsimd if (GPSIMD_ADD1 and hf == 0) else nc.vector
                eng.tensor_add(a1[:], t[:, 0:mh * dim // 2], t[:, mh * dim // 2:mh * dim])
                halves.append(a1)
            # merge halves then keep halving (all contiguous adds) down to [128, dim]
            n = mh * dim // 2
            cur = tree_p.tile([128, n], FP32, tag="m0")
            nc.vector.tensor_add(cur[:], halves[0][:], halves[1][:])
            while n > dim:
                n //= 2
                if n == dim:
                    nxt = p2_p.tile([128, dim], FP32, tag="p2", name="p2")
                else:
                    nxt = tree_p.tile([128, n], FP32, tag=f"m{n}", name=f"m{n}")
                nc.vector.tensor_add(nxt[:], cur[:, 0:n], cur[:, n:2 * n])
                cur = nxt
            # bucket-pair grouping (scaled 1/128 -> means), transposed to [d, b]
            sqt_ps = sqt_ps_p.tile([dim, B], FP32, tag="sqt_ps")
            nc.tensor.matmul(sqt_ps[:], lhsT=cur[:], rhs=g_sb[:], start=True, stop=True)
            sqt_sb = sqt_sb_p.tile([dim, B], FP32, tag="sqt_sb")
            nc.scalar.copy(sqt_sb[:], sqt_ps[:])
            sqt_tiles[(bh, name)] = sqt_sb
        # R block for this bh into the square; relu + ln per block (early blocks
        # run while later bh still stream)
        c, h = bh // 2, bh % 2
        nc.tensor.matmul(s_ps0[h * B:(h + 1) * B, c * B:(c + 1) * B],
                         lhsT=sqt_tiles[(bh, "q")][:],
                         rhs=sqt_tiles[(bh, "k")][:], start=True, stop=True)
        rrb = small_p.tile([B, B], FP32, tag="rrb")
        nc.vector.tensor_scalar_max(rrb[:], s_ps0[h * B:(h + 1) * B, c * B:(c + 1) * B], 0.0)
        nc.scalar.activation(ln_r[h * B:(h + 1) * B, c * B:(c + 1) * B], rrb[:],
                             AF.Ln, bias=eps_sb[0:B, :], scale=R_SCALE)

    # ---- P0 = exp((ln_r - t3) / T), and its transpose ----
    d0 = init_p.tile([S2, S2], FP32, tag="d0")
    nc.vector.tensor_sub(d0[:], ln_r[:], t3[:])
    p0_sb = init_p.tile([S2, S2], FP32, tag="p0")
    nc.scalar.activation(p0_sb[:], d0[:], AF.Exp, scale=INV_T)
    p0t_ps = s_ps_p.tile([S2, S2], FP32, tag="s_ps")
    nc.tensor.transpose(p0t_ps[:], p0_sb[:], id2b_sb[:])
    p0t_sb = init_p.tile([S2, S2], FP32, tag="p0t")
    nc.vector.tensor_copy(p0t_sb[:], p0t_ps[:])

    # ---- sinkhorn, vector form ----
    # step even (u-step): mv = P0 @ v     (lhsT = P0^T), next = recip(mv)*maskU
    # step odd  (v-step): mv = P0^T @ u   (lhsT = P0),   next = recip(mv)*maskV
    # rhs columns are (c', h'); valid entries selected by the masks.
    cur_rhs = maskv_sb          # v0 = ones, masked
    u_masked = v_masked = None
    for step in range(2 * SINKHORN_ITER):
        lhs = p0t_sb if step % 2 == 0 else p0_sb
        msk = masku_sb if step % 2 == 0 else maskv_sb
        mv = mv_ps_p.tile([S2, 4], FP32, tag="mv")
        nc.tensor.matmul(mv[:], lhsT=lhs[:], rhs=cur_rhs[:], start=True, stop=True)
        rec = small_p.tile([S2, 4], FP32, tag="rec")
        nc.vector.reciprocal(rec[:], mv[:])
        nxt = small_p.tile([S2, 4], FP32, tag=f"nx{step % 3}")
        nc.vector.tensor_mul(nxt[:], rec[:], msk[:])
        cur_rhs = nxt
        if step == 2 * SINKHORN_ITER - 2:
            u_masked = nxt
        elif step == 2 * SINKHORN_ITER - 1:
            v_masked = nxt

    # compact scale vectors:
    # u2[(h,i), c] = sum_{h'} u_masked[:, (c, h')]   (only h'==h nonzero)
    # v2[(c,j), h] = sum_{c'} v_masked[:, (c', h)]   (only c'==c nonzero)
    um4 = u_masked[:].rearrange("p (c h) -> p c h", c=2)
    u2 = small_p.tile([S2, 2], FP32, tag="u2")
    nc.vector.tensor_add(u2[:], um4[:, :, 0], um4[:, :, 1])
    v2 = small_p.tile([S2, 2], FP32, tag="v2")
    nc.vector.tensor_add(v2[:], v_masked[:, 0:2], v_masked[:, 2:4])

    # ---- final P = diag(u) P0 diag(v): scale P0^T by v, transpose, scale by u
    xv = init_p.tile([S2, S2], FP32, tag="xv")
    nc.vector.tensor_scalar_mul(xv[:, 0:B], p0t_sb[:, 0:B], v2[:, 0:1])
    nc.scalar.activation(xv[:, B:S2], p0t_sb[:, B:S2], AF.Copy, scale=v2[:, 1:2])
    xt_ps = s_ps_p.tile([S2, S2], FP32, tag="s_ps")
    nc.tensor.transpose(xt_ps[:], xv[:], id2b_sb[:])
    s_fin = out_p.tile([S2, S2], FP32, tag="s_fin")
    nc.vector.tensor_scalar_mul(s_fin[:, 0:B], xt_ps[:, 0:B], u2[:, 0:1])
    nc.scalar.activation(s_fin[:, B:S2], xt_ps[:, B:S2], AF.Copy, scale=u2[:, 1:2])
    nc.sync.dma_start(out.rearrange("(c h) i j -> h i c j", c=2, h=2), s_fin[:])


def build(bh_per: int, seq: int, dim: int):
    nc = bacc.Bacc(
        "TRN2",
        target_bir_lowering=False,
        debug=False,
        enable_asserts=True,
        num_devices=1,
    )
    B = seq // BUCKET
    aps = {}
    aps["q"] = nc.dram_tensor("q", (bh_per, seq, dim), FP32, kind="ExternalInput").ap()
    aps["k"] = nc.dram_tensor("k", (bh_per, seq, dim), FP32, kind="ExternalInput").ap()
    aps["u"] = nc.dram_tensor("u", (bh_per, B, B), FP32, kind="ExternalInput").ap()
    aps["g"] = nc.dram_tensor("g", (128, B), FP32, kind="ExternalInput").ap()
    aps["id2b"] = nc.dram_tensor("id2b", (2 * B, 2 * B), FP32, kind="ExternalInput").ap()
    aps["masku"] = nc.dram_tensor("masku", (2 * B, 4), FP32, kind="ExternalInput").ap()
    aps["maskv"] = nc.dram_tensor("maskv", (2 * B, 4), FP32, kind="ExternalInput").ap()
    aps["out"] = nc.dram_tensor("out", (bh_per, B, B), FP32, kind="ExternalOutput").ap()
    with tile.TileContext(nc) as tc:
        with ExitStack() as ctx:
            _emit(ctx, tc, aps, bh_per, seq, dim)
    nc.compile()
    return nc


def make_consts(seq: int):
    B = seq // BUCKET
    r0 = seq // 128
    ppb = BUCKET // r0  # partitions per bucket
    g = np.zeros((128, B), dtype=np.float32)
    for p in range(128):
        g[p, p // ppb] = 1.0 / BUCKET
    id2b = np.eye(2 * B, dtype=np.float32)
    # masks [S2, 4]; columns indexed (c', h') = c'*2 + h'
    masku = np.zeros((2 * B, 4), dtype=np.float32)  # (h == h')
    maskv = np.zeros((2 * B, 4), dtype=np.float32)  # (c == c')
    for p in range(2 * B):
        half = p // B
        for cp in range(2):
            for hp in range(2):
                col = cp * 2 + hp
                if hp == half:
                    masku[p, col] = 1.0
                if cp == half:
                    maskv[p, col] = 1.0
    return g, id2b, masku, maskv


_CACHE: dict = {}


def _get_nc(bh_per: int, seq: int, dim: int):
    key = (bh_per, seq, dim)
    if key not in _CACHE:
        _CACHE[key] = build(bh_per, seq, dim)
    return _CACHE[key]


def run(q, k, u_gumbel, topk=1, trace=False):
    q = np.ascontiguousarray(np.asarray(q), dtype=np.float32)
    k = np.ascontiguousarray(np.asarray(k), dtype=np.float32)
    u = np.ascontiguousarray(np.asarray(u_gumbel), dtype=np.float32)
    bh, seq, dim = q.shape
    per = bh // N_CORES
    nc = _get_nc(per, seq, dim)
    g, id2b, masku, maskv = make_consts(seq)
    in_maps = []
    for c in range(N_CORES):
        sl = slice(c * per, (c + 1) * per)
        in_maps.append({
            "q": q[sl], "k": k[sl], "u": u[sl],
            "g": g, "id2b": id2b, "masku": masku, "maskv": maskv,
        })
    res = run_bass_kernel_spmd(nc, in_maps, core_ids=list(range(N_CORES)), trace=trace)
    out = np.concatenate([res.results[c]["out"] for c in range(N_CORES)], axis=0)
    return out, res


def kernel(q, k, u_gumbel, topk=1):
    out, _ = run(q, k, u_gumbel, topk)
    return out


# revision 11
# speedup vs baseline: 1.5234x; 1.0283x over previous
"""AttentionSortNet Trainium2 kernel.

kernel(q, k, u_gumbel, topk) -> [bh, buckets, kv_buckets] f32

Sharding: bh (batch*heads) axis split across 8 NeuronCores, embarrassingly
data-parallel. Per core (bh_per=4): stream q/k shards (16MB, HWDGE), bucket
sums via a contiguous halving add-tree (DVE, + one gpsimd level), a small
grouping matmul produces transposed bucket means, R = sq @ sk.T * dim**-0.5,
gumbel perturbation, then 8 sinkhorn iterations in *vector form*
(u = 1/(P0 v), v = 1/(P0^T u) via PE matvecs; mathematically the same
iteration as the reference's log-space normalization), final
P = diag(u) P0 diag(v).
"""

import numpy as np
from contextlib import ExitStack

import concourse.bass as bass
import concourse.tile as tile
from concourse import bacc, mybir
from concourse.bass_utils import run_bass_kernel_spmd

FP32 = mybir.dt.float32
AF = mybir.ActivationFunctionType
ALU = mybir.AluOpType
AX = mybir.AxisListType

N_CORES = 8
BUCKET = 128  # rows per bucket (both q and kv side)
EPS = 1e-6
INV_T = 1.0 / 0.7
R_SCALE = 0.125  # DIM ** -0.5 for DIM=64
SINKHORN_ITER = 8
GPSIMD_ADD1 = True  # first fold level of each tile's first half runs on gpsimd


def _emit(ctx: ExitStack, tc: "tile.TileContext", aps: dict, bh_per: int, seq: int, dim: int):
    nc = tc.nc
    q, k, u, g, id2b, masku, maskv, out = (
        aps["q"], aps["k"], aps["u"], aps["g"], aps["id2b"],
        aps["masku"], aps["maskv"], aps["out"])
    B = seq // BUCKET            # buckets (= kv_buckets)
    r0 = seq // 128              # rows of one bh handled per partition
    mh = r0 // 2                 # rows per DMA half
    assert r0 % 2 == 0 and bh_per == 4 and B in (32, 64) and dim == 64
    S2 = 2 * B                   # square side: partition=(h,i), free=(c,j)

    const_p = ctx.enter_context(tc.tile_pool(name="const", bufs=1))
    acc_p = ctx.enter_context(tc.tile_pool(name="acc", bufs=10))
    tree_p = ctx.enter_context(tc.tile_pool(name="tree", bufs=4))
    p2_p = ctx.enter_context(tc.tile_pool(name="p2", bufs=4))
    sqt_sb_p = ctx.enter_context(tc.tile_pool(name="sqt_sb", bufs=2 * bh_per))
    sqt_ps_p = ctx.enter_context(tc.tile_pool(name="sqt_ps", bufs=2, space="PSUM"))
    init_p = ctx.enter_context(tc.tile_pool(name="init", bufs=2))
    small_p = ctx.enter_context(tc.tile_pool(name="small", bufs=8))
    s_ps_p = ctx.enter_context(tc.tile_pool(name="s_ps", bufs=2, space="PSUM"))
    mv_ps_p = ctx.enter_context(tc.tile_pool(name="mv_ps", bufs=3, space="PSUM"))
    out_p = ctx.enter_context(tc.tile_pool(name="outsb", bufs=2))

    # ---- constants (ACT-ring DMAs; keep the SP ring for bulk) ----
    eps_sb = const_p.tile([128, 1], FP32)
    nc.gpsimd.memset(eps_sb[:], EPS)
    g_sb = const_p.tile([128, B], FP32)
    nc.scalar.dma_start(g_sb[:], g)
    id2b_sb = const_p.tile([S2, S2], FP32)
    nc.scalar.dma_start(id2b_sb[:], id2b)
    masku_sb = const_p.tile([S2, 4], FP32)
    nc.scalar.dma_start(masku_sb[:], masku)
    maskv_sb = const_p.tile([S2, 4], FP32)
    nc.scalar.dma_start(maskv_sb[:], maskv)
    # u in "square" layout: partition = (h, i), free = (c, j), bh = 2c + h
    u_sq = const_p.tile([S2, S2], FP32)
    nc.scalar.dma_start(u_sq[:], u.rearrange("(c h) i j -> h i c j", c=2, h=2))

    # gumbel path early (independent of q/k): t3 = ln(-ln(u + eps) + eps)
    t1 = init_p.tile([S2, S2], FP32, tag="t1")
    nc.scalar.activation(t1[:], u_sq[:], AF.Ln, bias=eps_sb[0:S2, :], scale=1.0)
    t3 = init_p.tile([S2, S2], FP32, tag="t3")
    nc.scalar.activation(t3[:], t1[:], AF.Ln, bias=eps_sb[0:S2, :], scale=-1.0)

    # ---- bulk streaming + bucket sums ----
    # row of bh-slice = p*r0 + h*mh + m  (p: partition, h: half, m: row in half)
    qv = q.rearrange("b (p h m) d -> b h p (m d)", p=128, h=2, m=mh)
    kv = k.rearrange("b (p h m) d -> b h p (m d)", p=128, h=2, m=mh)

    s_ps0 = s_ps_p.tile([S2, S2], FP32, tag="s_ps")   # R square
    ln_r = init_p.tile([S2, S2], FP32, tag="lnr")
    sqt_tiles = {}
    for bh in range(bh_per):
        for name, src in (("q", qv), ("k", kv)):
            halves = []
            for hf in range(2):
                t = acc_p.tile([128, mh * dim], FP32, tag="acc")
                nc.sync.dma_start(t[:], src[bh, hf])
                a1 = tree_p.tile([128, mh * dim // 2], FP32, tag="a1")
                eng = nc.gpsimd if (GPSIMD_ADD1 and hf == 0) else nc.vector
                eng.tensor_add(a1[:], t[:, 0:mh * dim // 2], t[:, mh * dim // 2:mh * dim])
                halves.append(a1)
            # merge halves then keep halving (all contiguous adds) down to
            # [128, 2*dim]. A level whose output is 512 f32 writes split at a
            # 2KB gap: 0x400-apart DVE source pairs read ~7x slower.
            def add_level(srcs, out_n, tag):
                if out_n == 512:
                    t = tree_p.tile([128, 1024], FP32, tag=tag, name=tag)
                    dst = t[:].rearrange("p (a c) -> p a c", a=2)[:, :, 0:256]
                    rd = (t[:, 0:256], t[:, 512:768])
                elif out_n == 2 * dim:
                    t = p2_p.tile([128, out_n], FP32, tag="p2", name="p2")
                    dst = t[:]
                    rd = (t[:, 0:out_n // 2], t[:, out_n // 2:out_n])
                else:
                    t = tree_p.tile([128, out_n], FP32, tag=tag, name=tag)
                    dst = t[:]
                    rd = (t[:, 0:out_n // 2], t[:, out_n // 2:out_n])
                nc.vector.tensor_add(dst, srcs[0], srcs[1])
                return t, rd

            n = mh * dim // 2
            cur_t, cur_rd = add_level((halves[0][:], halves[1][:]), n, "m0")
            while n > 2 * dim:
                n //= 2
                cur_t, cur_rd = add_level(cur_rd, n, f"m{n}")
            p2 = cur_t
            # bucket-pair grouping (scaled 1/128 -> means), transposed to [d, b];
            # the (m2) pair fold is absorbed by accumulating two matmuls
            sqt_ps = sqt_ps_p.tile([dim, B], FP32, tag="sqt_ps")
            nc.tensor.matmul(sqt_ps[:], lhsT=p2[:, 0:dim], rhs=g_sb[:], start=True, stop=False)
            nc.tensor.matmul(sqt_ps[:], lhsT=p2[:, dim:2 * dim], rhs=g_sb[:], start=False, stop=True)
            sqt_sb = sqt_sb_p.tile([dim, B], FP32, tag="sqt_sb")
            nc.scalar.copy(sqt_sb[:], sqt_ps[:])
            sqt_tiles[(bh, name)] = sqt_sb
        # R block for this bh into the square; relu + ln per block (early blocks
        # run while later bh still stream)
        c, h = bh // 2, bh % 2
        nc.tensor.matmul(s_ps0[h * B:(h + 1) * B, c * B:(c + 1) * B],
                         lhsT=sqt_tiles[(bh, "q")][:],
                         rhs=sqt_tiles[(bh, "k")][:], start=True, stop=True)
        rrb = small_p.tile([B, B], FP32, tag="rrb")
        nc.vector.tensor_scalar_max(rrb[:], s_ps0[h * B:(h + 1) * B, c * B:(c + 1) * B], 0.0)
        nc.scalar.activation(ln_r[h * B:(h + 1) * B, c * B:(c + 1) * B], rrb[:],
                             AF.Ln, bias=eps_sb[0:B, :], scale=R_SCALE)

    # ---- P0 = exp((ln_r - t3) / T), and its transpose ----
    d0 = init_p.tile([S2, S2], FP32, tag="d0")
    nc.vector.tensor_sub(d0[:], ln_r[:], t3[:])
    p0_sb = init_p.tile([S2, S2], FP32, tag="p0")
    nc.scalar.activation(p0_sb[:], d0[:], AF.Exp, scale=INV_T)
    p0t_ps = s_ps_p.tile([S2, S2], FP32, tag="s_ps")
    nc.tensor.transpose(p0t_ps[:], p0_sb[:], id2b_sb[:])
    p0t_sb = init_p.tile([S2, S2], FP32, tag="p0t")
    nc.vector.tensor_copy(p0t_sb[:], p0t_ps[:])

    # ---- sinkhorn, vector form ----
    # step even (u-step): mv = P0 @ v     (lhsT = P0^T), next = recip(mv)*maskU
    # step odd  (v-step): mv = P0^T @ u   (lhsT = P0),   next = recip(mv)*maskV
    # rhs columns are (c', h'); valid entries selected by the masks.
    cur_rhs = maskv_sb          # v0 = ones, masked
    u_masked = v_masked = None
    for step in range(2 * SINKHORN_ITER):
        lhs = p0t_sb if step % 2 == 0 else p0_sb
        msk = masku_sb if step % 2 == 0 else maskv_sb
        mv = mv_ps_p.tile([S2, 4], FP32, tag="mv")
        nc.tensor.matmul(mv[:], lhsT=lhs[:], rhs=cur_rhs[:], start=True, stop=True)
        rec = small_p.tile([S2, 4], FP32, tag="rec")
        nc.vector.reciprocal(rec[:], mv[:])
        nxt = small_p.tile([S2, 4], FP32, tag=f"nx{step % 3}")
        nc.vector.tensor_mul(nxt[:], rec[:], msk[:])
        cur_rhs = nxt
        if step == 2 * SINKHORN_ITER - 2:
            u_masked = nxt
        elif step == 2 * SINKHORN_ITER - 1:
            v_masked = nxt

    # compact scale vectors:
    # u2[(h,i), c] = sum_{h'} u_masked[:, (c, h')]   (only h'==h nonzero)
    # v2[(c,j), h] = sum_{c'} v_masked[:, (c', h)]   (only c'==c nonzero)
    um4 = u_masked[:].rearrange("p (c h) -> p c h", c=2)
    u2 = small_p.tile([S2, 2], FP32, tag="u2")
    nc.vector.tensor_add(u2[:], um4[:, :, 0], um4[:, :, 1])
    v2 = small_p.tile([S2, 2], FP32, tag="v2")
    nc.vector.tensor_add(v2[:], v_masked[:, 0:2], v_masked[:, 2:4])

    # ---- final P = diag(u) P0 diag(v): scale P0^T by v, transpose, scale by u
    xv = init_p.tile([S2, S2], FP32, tag="xv")
    nc.vector.tensor_scalar_mul(xv[:, 0:B], p0t_sb[:, 0:B], v2[:, 0:1])
    nc.scalar.activation(xv[:, B:S2], p0t_sb[:, B:S2], AF.Copy, scale=v2[:, 1:2])
    xt_ps = s_ps_p.tile([S2, S2], FP32, tag="s_ps")
    nc.tensor.transpose(xt_ps[:], xv[:], id2b_sb[:])
    s_fin = out_p.tile([S2, S2], FP32, tag="s_fin")
    nc.vector.tensor_scalar_mul(s_fin[:, 0:B], xt_ps[:, 0:B], u2[:, 0:1])
    nc.scalar.activation(s_fin[:, B:S2], xt_ps[:, B:S2], AF.Copy, scale=u2[:, 1:2])
    nc.sync.dma_start(out.rearrange("(c h) i j -> h i c j", c=2, h=2), s_fin[:])


def build(bh_per: int, seq: int, dim: int):
    nc = bacc.Bacc(
        "TRN2",
        target_bir_lowering=False,
        debug=False,
        enable_asserts=True,
        num_devices=1,
    )
    B = seq // BUCKET
    aps = {}
    aps["q"] = nc.dram_tensor("q", (bh_per, seq, dim), FP32, kind="ExternalInput").ap()
    aps["k"] = nc.dram_tensor("k", (bh_per, seq, dim), FP32, kind="ExternalInput").ap()
    aps["u"] = nc.dram_tensor("u", (bh_per, B, B), FP32, kind="ExternalInput").ap()
    aps["g"] = nc.dram_tensor("g", (128, B), FP32, kind="ExternalInput").ap()
    aps["id2b"] = nc.dram_tensor("id2b", (2 * B, 2 * B), FP32, kind="ExternalInput").ap()
    aps["masku"] = nc.dram_tensor("masku", (2 * B, 4), FP32, kind="ExternalInput").ap()
    aps["maskv"] = nc.dram_tensor("maskv", (2 * B, 4), FP32, kind="ExternalInput").ap()
    aps["out"] = nc.dram_tensor("out", (bh_per, B, B), FP32, kind="ExternalOutput").ap()
    with tile.TileContext(nc) as tc:
        with ExitStack() as ctx:
            _emit(ctx, tc, aps, bh_per, seq, dim)
    nc.compile()
    return nc


def make_consts(seq: int):
    B = seq // BUCKET
    r0 = seq // 128
    ppb = BUCKET // r0  # partitions per bucket
    g = np.zeros((128, B), dtype=np.float32)
    for p in range(128):
        g[p, p // ppb] = 1.0 / BUCKET
    id2b = np.eye(2 * B, dtype=np.float32)
    # masks [S2, 4]; columns indexed (c', h') = c'*2 + h'
    masku = np.zeros((2 * B, 4), dtype=np.float32)  # (h == h')
    maskv = np.zeros((2 * B, 4), dtype=np.float32)  # (c == c')
    for p in range(2 * B):
        half = p // B
        for cp in range(2):
            for hp in range(2):
                col = cp * 2 + hp
                if hp == half:
                    masku[p, col] = 1.0
                if cp == half:
                    maskv[p, col] = 1.0
    return g, id2b, masku, maskv


_CACHE: dict = {}


def _get_nc(bh_per: int, seq: int, dim: int):
    key = (bh_per, seq, dim)
    if key not in _CACHE:
        _CACHE[key] = build(bh_per, seq, dim)
    return _CACHE[key]


def run(q, k, u_gumbel, topk=1, trace=False):
    q = np.ascontiguousarray(np.asarray(q), dtype=np.float32)
    k = np.ascontiguousarray(np.asarray(k), dtype=np.float32)
    u = np.ascontiguousarray(np.asarray(u_gumbel), dtype=np.float32)
    bh, seq, dim = q.shape
    per = bh // N_CORES
    nc = _get_nc(per, seq, dim)
    g, id2b, masku, maskv = make_consts(seq)
    in_maps = []
    for c in range(N_CORES):
        sl = slice(c * per, (c + 1) * per)
        in_maps.append({
            "q": q[sl], "k": k[sl], "u": u[sl],
            "g": g, "id2b": id2b, "masku": masku, "maskv": maskv,
        })
    res = run_bass_kernel_spmd(nc, in_maps, core_ids=list(range(N_CORES)), trace=trace)
    out = np.concatenate([res.results[c]["out"] for c in range(N_CORES)], axis=0)
    return out, res


def kernel(q, k, u_gumbel, topk=1):
    out, _ = run(q, k, u_gumbel, topk)
    return out
